# revision 1
# baseline (speedup 1.0000x reference)
"""Trainium2 Bass kernel: multi-head attention (B=4, S=2048, E=1024, H=16, D=64).

Sharding: 8 cores = 4 batches x 2 head-groups (8 heads each). Each core
computes attention for its (batch, 8-head group) and a partial output
projection over its 512 channels; the host sums the two partials per batch
and adds the output bias.

Per-core dataflow (transposed formulation, bf16 matmuls, fp32 PSUM):
  XT_aug[h]  = [x[b].T rows for head h ; ones]      (65, S)   bf16  (host-prepped)
  QT[h]      = Wq_aug[h].T @ XT_aug[h]              (64, S)   via PE, bias via ones row
  KT[h]      likewise
  V_aug[h,t] = XT_aug[h][:,t128].T @ Wv_ext[h]      (128, 65) last col = ones
  ST[t,s]    = KT-tile.T @ QT                       scores^T in PSUM
  expT       = exp(0.125 * ST)                      ScalarE, PSUM->SBUF bf16
  outT[h]    = sum_t V_aug[h,t].T @ expT[t]         (65, S); row 64 = softmax denom
  concatT    = outT[0:64] * (1/denom)               bcast via DRAM bounce
  partial    = concatT.T @ WoT                      (S, 1024) fp32 -> DRAM
"""

import sys

sys.path.insert(0, "/opt/trn_rl_repo")

import numpy as np
import ml_dtypes

BF16 = ml_dtypes.bfloat16

B, S, E, H = 4, 2048, 1024, 16
D = E // H          # 64
HL = 8              # heads per core
N_CORES = 8
SB = 512            # psum-bank-sized score/attnV block (fp32)
EXP_W = 1024        # exp tile width (2 psum banks)

_CACHE = {}


def build_nc(s=S):
    import concourse.bass as bass
    import concourse.mybir as mybir
    import concourse.tile as tile
    from concourse import bacc

    f32 = mybir.dt.float32
    bf16 = mybir.dt.bfloat16
    sb = min(SB, s)
    expw = min(EXP_W, s)
    n_tt = s // 128          # t tiles
    n_sb = s // sb           # attnV / score column blocks
    n_et = s // expw         # exp tiles per t-tile
    sb_per_et = expw // sb

    nc = bacc.Bacc(None)

    xt_d = nc.dram_tensor("xt", [HL, 128, s], bf16, kind="ExternalInput")
    wq_d = nc.dram_tensor("wq", [HL, 128, 128], bf16, kind="ExternalInput")
    wk_d = nc.dram_tensor("wk", [HL, 128, 128], bf16, kind="ExternalInput")
    wv_d = nc.dram_tensor("wv", [HL, 128, D + 1], bf16, kind="ExternalInput")
    wot_d = nc.dram_tensor("wot", [HL * D, E], bf16, kind="ExternalInput")
    out_d = nc.dram_tensor("out", [s, E], f32, kind="ExternalOutput")
    recip_d = nc.dram_tensor("recip_dram", [s], f32)

    with tile.TileContext(nc) as tc:
        with (
            tc.tile_pool(name="xt", bufs=HL) as xt_pool,
            tc.tile_pool(name="w", bufs=3 * HL) as w_pool,
            tc.tile_pool(name="qt", bufs=HL) as qt_pool,
            tc.tile_pool(name="kt", bufs=HL) as kt_pool,
            tc.tile_pool(name="v", bufs=HL) as v_pool,
            tc.tile_pool(name="wot", bufs=4) as wot_pool,
            tc.tile_pool(name="et", bufs=4) as et_pool,
            tc.tile_pool(name="ot", bufs=2) as ot_pool,
            tc.tile_pool(name="norm", bufs=2) as norm_pool,
            tc.tile_pool(name="ct", bufs=4) as ct_pool,
            tc.tile_pool(name="ctmp", bufs=2) as ctmp_pool,
        ):
            # ---- load weights + XT ----
            xts, wqs, wks, wvs = [], [], [], []
            qts, kts, vs = [None] * HL, [None] * HL, [None] * HL
            for j in range(HL):
                xtj = xt_pool.tile([128, s], bf16, tag="xt")
                nc.sync.dma_start(out=xtj[:, :], in_=xt_d[j])
                xts.append(xtj)
                wqj = w_pool.tile([128, 128], bf16, tag="w")
                wkj = w_pool.tile([128, 128], bf16, tag="w")
                wvj = w_pool.tile([128, D + 1], bf16, tag="w")
                nc.sync.dma_start(out=wqj[:, :], in_=wq_d[j])
                nc.sync.dma_start(out=wkj[:, :], in_=wk_d[j])
                nc.sync.dma_start(out=wvj[:, :], in_=wv_d[j])
                wqs.append(wqj)
                wks.append(wkj)
                wvs.append(wvj)
            wots = []
            for p in range(4):
                wt = wot_pool.tile([128, E], bf16, tag="wot")
                nc.sync.dma_start(out=wt[:, :], in_=wot_d[p * 128 : (p + 1) * 128, :])
                wots.append(wt)

            # ---- attention per head, s in two half-passes (halves the PSUM
            # residency of the attnV accumulators), next head's QKV
            # software-pipelined into dedicated PSUM banks ----
            cts = [ct_pool.tile([128, s], bf16, tag="ct", name=f"ct{p}")
                   for p in range(HL // 2)]
            n_pass = max(1, s // expw)       # s-half passes per head
            sb_per_pass = n_sb // n_pass     # attnV accumulators per pass
            with (
                tc.tile_pool(name="sc_ps", bufs=2, space="PSUM") as sc_ps,
                tc.tile_pool(name="av_ps", bufs=2, space="PSUM") as av_ps,
                tc.tile_pool(name="qkv_ps", bufs=2, space="PSUM") as qkv_ps,
            ):
                def emit_qkv_item(j, item):
                    """item 0..n_sb*2-1: q/k block; item >= n_sb*2: v tile."""
                    if item == 0:
                        qts[j] = qt_pool.tile([128, s], bf16, tag="qt",
                                              name=f"qt{j}")
                        kts[j] = kt_pool.tile([128, s], bf16, tag="kt",
                                              name=f"kt{j}")
                        vs[j] = v_pool.tile([128, n_tt * (D + 1)], bf16,
                                            tag="v", name=f"v{j}")
                    if item < n_sb * 2:
                        blk, which = divmod(item, 2)
                        sl = slice(blk * sb, (blk + 1) * sb)
                        w = wqs[j] if which == 0 else wks[j]
                        dst = qts[j] if which == 0 else kts[j]
                        pool = sc_ps if (j == 0 and item % 2 == 0) else qkv_ps
                        psq = pool.tile([128, sb], f32,
                                        tag="sc" if pool is sc_ps else "qkv",
                                        name=f"qk{j}_{item}")
                        nc.tensor.matmul(psq[:, :], w[:, :], xts[j][:, sl])
                        nc.vector.tensor_copy(dst[:, sl], psq[:, :])
                    else:
                        tt = item - n_sb * 2
                        tsl = slice(tt * 128, (tt + 1) * 128)
                        psv = qkv_ps.tile([128, D + 1], f32, tag="qkv",
                                          name=f"v{j}_{tt}")
                        nc.tensor.matmul(psv[:, :], xts[j][:, tsl], wvs[j][:, :])
                        nc.vector.tensor_copy(
                            vs[j][:, tt * (D + 1) : (tt + 1) * (D + 1)],
                            psv[:, :],
                        )

                n_items = n_sb * 2 + n_tt
                for it in range(n_items):
                    emit_qkv_item(0, it)
                head_order = list(range(HL - 2)) + [HL - 1, HL - 2]
                for jpos, j in enumerate(head_order):
                    oT = ot_pool.tile([D + 1, s], f32, tag="ot", name=f"oT{j}")
                    nj = head_order[jpos + 1] if jpos + 1 < HL else None
                    nxt = list(range(n_items)) if nj is not None else []
                    n_slots = n_pass * n_tt
                    per_slot = (len(nxt) + n_slots - 1) // n_slots if nxt else 0
                    for pass_ in range(n_pass):
                        avs = [
                            av_ps.tile([D + 1, sb], f32, tag="av",
                                       name=f"av{j}_{pass_}_{k}")
                            for k in range(sb_per_pass)
                        ]
                        for tt in range(n_tt):
                            ksl = kts[j][:, tt * 128 : (tt + 1) * 128]
                            vsl = vs[j][:, tt * (D + 1) : (tt + 1) * (D + 1)]
                            ps = sc_ps.tile([128, expw], f32, tag="sc")
                            for k in range(sb_per_pass):
                                blk = pass_ * sb_per_pass + k
                                nc.tensor.matmul(
                                    ps[:, k * sb : (k + 1) * sb],
                                    ksl,
                                    qts[j][:, blk * sb : (blk + 1) * sb],
                                )
                            et = et_pool.tile([128, expw], bf16, tag="et")
                            nc.scalar.activation(
                                et[:, :],
                                ps[:, :],
                                mybir.ActivationFunctionType.Exp,
                                scale=float(1.0 / np.sqrt(D)),
                            )
                            for k in range(sb_per_pass):
                                nc.tensor.matmul(
                                    avs[k][:, :],
                                    vsl,
                                    et[:, k * sb : (k + 1) * sb],
                                    start=(tt == 0),
                                    stop=(tt == n_tt - 1),
                                )
                            for _ in range(per_slot):
                                if nxt:
                                    emit_qkv_item(nj, nxt.pop(0))
                        # release accumulators into oT
                        for k in range(sb_per_pass):
                            blk = pass_ * sb_per_pass + k
                            nc.vector.tensor_copy(
                                oT[:, blk * sb : (blk + 1) * sb], avs[k][:, :]
                            )
                    # normalization: recip of colsum (row 64), bcast via DRAM
                    cs128 = norm_pool.tile([128, s // 128], f32, tag="cs")
                    rc128 = norm_pool.tile([128, s // 128], f32, tag="rc")
                    bcast = norm_pool.tile([D, s], f32, tag="bc")
                    nc.sync.dma_start(out=cs128[:, :], in_=oT[D : D + 1, :])
                    nc.vector.reciprocal(rc128[:, :], cs128[:, :])
                    nc.sync.dma_start(out=recip_d[:], in_=rc128[:, :])
                    nc.sync.dma_start(
                        out=bcast[:, :],
                        in_=recip_d[:].unsqueeze(0).broadcast_to((D, s)),
                    )
                    ct = cts[j // 2]
                    if j % 2 == 0:
                        dst = ct
                    else:
                        dst = ctmp_pool.tile([D, s], bf16, tag="ctmp")
                    nc.vector.tensor_mul(dst[0:D, :], oT[0:D, :], bcast[:, :])
                    if j % 2 == 1:
                        nc.sync.dma_start(out=ct[D : 2 * D, :], in_=dst[:, :])

            # ---- output projection ----
            with (
                tc.tile_pool(name="pj_ps", bufs=3, space="PSUM") as pj_ps,
                tc.tile_pool(name="po", bufs=4) as po_pool,
            ):
                for sc in range(s // 128):
                    pso = pj_ps.tile([128, E], f32, tag="pj")
                    for p in range(4):
                        for half in range(2):
                            hsl = slice(half * 512, (half + 1) * 512)
                            nc.tensor.matmul(
                                pso[:, hsl],
                                cts[p][:, sc * 128 : (sc + 1) * 128],
                                wots[p][:, hsl],
                                start=(p == 0),
                                stop=(p == 3),
                            )
                    osb = po_pool.tile([128, E], f32, tag="po")
                    nc.vector.tensor_copy(osb[:, :], pso[:, :])
                    nc.sync.dma_start(
                        out=out_d[sc * 128 : (sc + 1) * 128, :], in_=osb[:, :]
                    )

    nc.compile()
    return nc


def prep_inputs(token_encodings, Wq, Wk, Wv, bq, bk, bv, Wo, bo):
    """Build per-core input maps. Core c = b*2+g."""
    x = np.asarray(token_encodings, dtype=np.float32)
    maps = []
    for c in range(N_CORES):
        b, g = divmod(c, 2)
        xt_full = np.ascontiguousarray(x[b].T)  # (E, S)
        xt = np.zeros((HL, 128, S), dtype=BF16)
        wq_a = np.zeros((HL, 128, 128), dtype=BF16)
        wk_a = np.zeros((HL, 128, 128), dtype=BF16)
        wv_a = np.zeros((HL, 128, D + 1), dtype=BF16)
        for j in range(HL):
            h = g * HL + j
            xt[j, :D] = xt_full[h * D : (h + 1) * D].astype(BF16)
            xt[j, D] = np.float32(1.0)
            wq_a[j, :D, :D] = np.asarray(Wq[h], np.float32).astype(BF16)
            wq_a[j, D, :D] = np.asarray(bq[h], np.float32).astype(BF16)
            wk_a[j, :D, :D] = np.asarray(Wk[h], np.float32).astype(BF16)
            wk_a[j, D, :D] = np.asarray(bk[h], np.float32).astype(BF16)
            wv_a[j, :D, :D] = np.asarray(Wv[h], np.float32).astype(BF16)
            wv_a[j, D, :D] = np.asarray(bv[h], np.float32).astype(BF16)
            wv_a[j, D, D] = np.float32(1.0)
        wot = np.ascontiguousarray(
            np.asarray(Wo, np.float32)[:, g * 512 : (g + 1) * 512].T
        ).astype(BF16)
        maps.append({"xt": xt, "wq": wq_a, "wk": wk_a, "wv": wv_a, "wot": wot})
    return maps


def kernel(**inputs):
    from concourse.bass_utils import run_bass_kernel_spmd

    if "nc" not in _CACHE:
        _CACHE["nc"] = build_nc()
    nc = _CACHE["nc"]
    in_maps = prep_inputs(**inputs)
    res = run_bass_kernel_spmd(nc, in_maps, list(range(N_CORES)))
    bo_f = np.asarray(inputs["bo"], np.float32)
    out = np.empty((B, S, E), dtype=np.float32)
    for b in range(B):
        out[b] = res.results[2 * b]["out"] + res.results[2 * b + 1]["out"] + bo_f
    return out

